# revision 15
# baseline (speedup 1.0000x reference)
"""Trainium2 Bass kernel for nn_ConvblockNofrills (dense_cnn).

Reference computation (per batch b, output position l, channel d):
    gate[b,l,d,k] = tanh( sum_c x[b, l+K-1, c] * weights[d, c, k] )
    out[b,l,d]    = sum_k x[b, l+k, d] * gate[b,l,d,k]
with B=8, T=4096, C=D=512, K=7, L=T-K+1=4090.

Strategy: data-parallel across the 8 NeuronCores (one batch each).
Per core everything runs in transposed (channel, position) layout:
  - gates via bf16 matmul on TensorE (fp32 PSUM accumulation)
  - tanh on ScalarE (fp32-accurate spline), output bf16 to SBUF
  - 7-tap multiply/accumulate on VectorE in bf16, per 512-col l-tile

The PE stream is the roofline (896 matmuls x 518 cyc = 193 us at
2.4 GHz); everything else exists to keep its head/tail short.
Measured constraints this is built around:
  - ~7.6 us fixed framework preamble before any kernel instruction.
  - All dma_starts drain through ONE FIFO ring at ~200-360 GB/s
    (rate depends on contiguous run length; >=4KB runs are fastest),
    so descriptors are issued on the Sync queue in exact consumption
    order: w[k=0], then x quarter-columns for all 4 channel chunks,
    then the remaining w taps, then the last x quarter.
  - The PE clock ramps (~0.9 -> 2.4 GHz) over several us and
    re-throttles after >1us idle; warmup matmuls (memset on VectorE)
    bridge from ~7.4 us until the first operands land (~12.5 us).
  - Quad 0 runs k-outer (w taps stream in one 512KB descriptor per
    ~14 us of compute); its k=0 sweep runs lt-outer/dc-inner in
    single-PSUM-bank groups so matmuls start after just w0 + one x
    quarter. Quad 1 runs dc-outer so each output chunk's store
    streams out every ~12 us instead of bunching 2MB at the end; the
    very last chunk pipelines tanh->mul->add->store per l-tile.
Host side packs/casts inputs (part of sharding) and transposes the
per-core result back to the (B, L, C) fp32 output.
"""

import numpy as np
import ml_dtypes

import sys
for _p in ("/opt/trn_rl_repo", "/root/.axon_site/_ro/trn_rl_repo"):
    if _p not in sys.path:
        sys.path.append(_p)

B, T, C, K = 8, 4096, 512, 7
L = T - K + 1  # 4090
NCORES = 8
P = 128           # partitions
DC = C // P       # 4 channel chunks
NL = 512          # l-tile (one PSUM bank of fp32)
QUAD = 4          # l-tiles per quad
NQ = 2            # quads
XQ = 1024         # x load quarter (columns)
NWARM_PRE = 4     # warmup matmuls hoisted before the entry barrier
NWARM = 5         # warmup matmuls in the body

_cache = {}


def _build():
    import concourse.bass as bass  # noqa: F401
    import concourse.mybir as mybir
    import concourse.tile as tile
    from concourse import bacc

    bf16 = mybir.dt.bfloat16
    f32 = mybir.dt.float32
    Tanh = mybir.ActivationFunctionType.Tanh

    nc = bacc.Bacc("TRN2", target_bir_lowering=False, debug=False,
                   num_devices=NCORES)

    # Instructions hoisted into the per-engine preamble (pre entry
    # barrier) after the TileContext closes: the ~7.5us framework
    # preamble then overlaps the first input DMAs and the PE clock
    # ramp. (Same entry-block insertion point bacc itself uses for the
    # kernel barrier.)
    hoist = {"sync": [], "scalar": [], "vector": [], "tensor": []}

    xT_d = nc.dram_tensor("xT", [C, T], bf16, kind="ExternalInput")
    # wB[k, p, cc, d] = weights[d, cc*128+p, k]
    wB_d = nc.dram_tensor("wB", [K, P, DC, C], bf16, kind="ExternalInput")
    # outB[dc, p, l] = out[l, dc*128+p]
    outB_d = nc.dram_tensor("outB", [DC, P, L], bf16, kind="ExternalOutput")

    with tile.TileContext(nc) as tc:
        with (
            tc.tile_pool(name="wpool", bufs=1) as wpool,
            tc.tile_pool(name="xpool", bufs=1) as xpool,
            tc.tile_pool(name="gpool", bufs=8) as gpool,
            tc.tile_pool(name="apool", bufs=6) as apool,
            tc.tile_pool(name="ppool", bufs=4) as ppool,
            tc.tile_pool(name="psum", bufs=8, space="PSUM") as psum_pool,
        ):
            wsb = wpool.tile([P, K, DC, C], bf16, name="w")
            xsb = [xpool.tile([P, T], bf16, name=f"x_{cc}")
                   for cc in range(DC)]
            warm = wpool.tile([P, NL], bf16, name="warm")

            # One FIFO DMA ring drains everything in descriptor-arrival
            # order, but issue cost (~0.6us each) is serial per engine:
            # split the critical first descriptors across BOTH HWDGE
            # queues (Scalar's preamble clears slightly earlier) and
            # hoist them pre-barrier so data flows during the framework
            # preamble. Remaining loads issue in consumption order.
            def load_w(k, eng=None):
                return (eng or nc.sync).dma_start(wsb[:, k, :, :],
                                                  wB_d.ap()[k])

            def load_x(cc, q, eng=None):
                return (eng or nc.sync).dma_start(
                    xsb[cc][:, q * XQ:(q + 1) * XQ],
                    xT_d.ap()[cc * P:(cc + 1) * P, q * XQ:(q + 1) * XQ])

            hoist["scalar"].append(load_w(0, nc.scalar).ins.name)
            hoist["sync"].append(load_x(0, 0).ins.name)
            hoist["sync"].append(load_x(1, 0).ins.name)
            hoist["scalar"].append(load_x(2, 0, nc.scalar).ins.name)
            hoist["scalar"].append(load_x(3, 0, nc.scalar).ins.name)
            load_x(0, 1, nc.scalar)
            load_x(1, 1, nc.scalar)
            load_x(2, 1)
            load_x(3, 1)
            for cc in range(DC):
                load_x(cc, 2)
            for k in range(1, K):
                load_w(k)
            for cc in range(DC):
                load_x(cc, 3)

            # Warmup matmuls keep the PE clock ramping from ~5.7us
            # until real operands land. (The psum result is never read.)
            ms = nc.vector.memset(warm[:], 1.0)
            hoist["vector"].append(ms.ins.name)
            warm_ps = psum_pool.tile([P, NL], f32, tag="ps", name="warm_ps")
            for i in range(NWARM_PRE + NWARM):
                r = nc.tensor.matmul(warm_ps, warm[:, :P], warm,
                                     start=True,
                                     stop=(i == NWARM_PRE + NWARM - 1))
                if i < NWARM_PRE:
                    hoist["tensor"].append(r.ins.name)

            def mm_group(ps, k, dc, l0, nl):
                """One PSUM accumulation group: gate matmuls for
                (k, dc) over output cols [l0, l0+nl)."""
                for cc in range(DC):
                    nc.tensor.matmul(
                        ps[:, :nl],
                        wsb[:, k, cc, dc * P:(dc + 1) * P],
                        xsb[cc][:, l0 + K - 1: l0 + K - 1 + nl],
                        start=(cc == 0),
                        stop=(cc == DC - 1),
                    )

            def unit(lq, dc, k, acc, last=False):
                """Gates + tanh + per-l-tile mul/add for one (dc, k);
                returns the new accumulator tile."""
                q0 = lq * QUAD * NL
                qn = min(QUAD * NL, L - q0)
                ps = [psum_pool.tile([P, NL], f32, tag="ps",
                                     name=f"ps_{lq}_{dc}_{k}_{i}")
                      for i in range(QUAD)]
                # l-tile-outer: each PSUM group completes as early as
                # possible so the tanh chain pipelines tightly (and the
                # last unit's epilogue is short).
                for i in range(QUAD):
                    l0 = q0 + i * NL
                    nl = min(NL, L - l0)
                    mm_group(ps[i], k, dc, l0, nl)
                return tail_unit(lq, dc, k, acc, ps, last)

            def tail_unit(lq, dc, k, acc, ps, last):
                """tanh + mul(+add) per l-tile given the unit's psums."""
                q0 = lq * QUAD * NL
                g = gpool.tile([P, QUAD * NL], bf16, tag="g",
                               name=f"g_{lq}_{dc}_{k}")
                nxt = apool.tile([P, QUAD * NL], bf16, tag="acc",
                                 name=f"acc_{lq}_{dc}_{k}")
                for i in range(QUAD):
                    l0 = q0 + i * NL
                    nl = min(NL, L - l0)
                    o = i * NL
                    gs = g[:, o:o + nl]
                    nc.scalar.activation(gs, ps[i][:, :nl], Tanh)
                    xu = xsb[dc][:, l0 + k:l0 + k + nl]
                    if acc is None:
                        nc.vector.tensor_mul(nxt[:, o:o + nl], gs, xu)
                    else:
                        prod = ppool.tile([P, QUAD * NL], bf16, tag="prod",
                                          name=f"prod_{lq}_{dc}_{k}_{i}")
                        nc.vector.tensor_mul(prod[:, o:o + nl], gs, xu)
                        nc.vector.tensor_add(nxt[:, o:o + nl],
                                             acc[:, o:o + nl],
                                             prod[:, o:o + nl])
                    if last and i % 2 == 1:
                        # store per half-quad right after its adds: bigger
                        # contiguous runs than per-tile, still early
                        h0 = q0 + (i - 1) * NL
                        hn = min(2 * NL, L - h0)
                        oh = (i - 1) * NL
                        nc.sync.dma_start(outB_d.ap()[dc, :, h0:h0 + hn],
                                          nxt[:, oh:oh + hn])
                return nxt

            def store(lq, dc, acc):
                q0 = lq * QUAD * NL
                qn = min(QUAD * NL, L - q0)
                nc.sync.dma_start(outB_d.ap()[dc, :, q0:q0 + qn],
                                  acc[:, :qn])

            # ---- Quad 0: k outer. k=0 runs lt-outer/dc-inner in
            # single-bank groups so the PE starts on minimal data.
            acc0 = [None] * DC
            ps00 = {}
            for i in range(QUAD):
                l0 = i * NL
                for dc in range(DC):
                    ps = psum_pool.tile([P, NL], f32, tag="ps",
                                        name=f"ps00_{i}_{dc}")
                    mm_group(ps, 0, dc, l0, NL)
                    ps00[(i, dc)] = ps
                    g = gpool.tile([P, NL], bf16, tag="g0",
                                   name=f"g00_{i}_{dc}")
                    nc.scalar.activation(g[:], ps[:], Tanh)
                    if acc0[dc] is None:
                        acc0[dc] = apool.tile([P, QUAD * NL], bf16,
                                              tag="acc", name=f"acc00_{dc}")
                    nc.vector.tensor_mul(acc0[dc][:, l0:l0 + NL], g[:],
                                         xsb[dc][:, l0:l0 + NL])
            for k in range(1, K):
                for dc in range(DC):
                    acc0[dc] = unit(0, dc, k, acc0[dc])
            for dc in range(DC):
                store(0, dc, acc0[dc])

            # ---- Quad 1: dc outer, k inner; stores stream per dc.
            for dc in range(DC):
                acc = None
                for k in range(K):
                    last = (dc == DC - 1 and k == K - 1)
                    acc = unit(1, dc, k, acc, last=last)
                if dc != DC - 1:
                    store(1, dc, acc)

    # Hoist the captured instructions into the entry-block preamble,
    # right after each issuing engine's preamble_end.
    blocks = nc.main_func.blocks
    entry = blocks[0]

    def move_pre_barrier(names, pend):
        order = {n: i for i, n in enumerate(names)}
        moved = []
        for blk in blocks[1:]:
            keep = []
            for inst in list(blk.instructions):
                if getattr(inst, "name", "") in order:
                    moved.append(inst)
                else:
                    keep.append(inst)
            if len(keep) != len(blk.instructions):
                blk.instructions = keep
        assert len(moved) == len(names), (len(moved), names)
        moved.sort(key=lambda i: order[i.name])
        idx = entry.instructions.index(pend) + 1
        for j, inst in enumerate(moved):
            entry.instructions.insert(idx + j, inst)

    move_pre_barrier(hoist["sync"], nc.sync.preamble_end)
    move_pre_barrier(hoist["scalar"], nc.scalar.preamble_end)
    move_pre_barrier(hoist["vector"], nc.vector.preamble_end)
    move_pre_barrier(hoist["tensor"], nc.tensor.preamble_end)

    nc.compile()
    return nc


def _prep_inputs(x, weights):
    bf = ml_dtypes.bfloat16
    # wB[k, p, cc, d] = weights[d, cc*128+p, k]
    wB = np.ascontiguousarray(
        weights.reshape(C, DC, P, K).transpose(3, 2, 1, 0)).astype(bf)
    in_maps = []
    for b in range(B):
        xT = np.ascontiguousarray(x[b].T).astype(bf)  # (C, T)
        in_maps.append({"xT": xT, "wB": wB})
    return in_maps


def kernel(x, weights):
    x = np.asarray(x, dtype=np.float32)
    weights = np.asarray(weights, dtype=np.float32)
    assert x.shape == (B, T, C) and weights.shape == (C, C, K)

    from concourse.bass_utils import run_bass_kernel_spmd

    if "nc" not in _cache:
        _cache["nc"] = _build()
    nc = _cache["nc"]

    in_maps = _prep_inputs(x, weights)
    res = run_bass_kernel_spmd(nc, in_maps, list(range(NCORES)))

    out = np.empty((B, L, C), dtype=np.float32)
    for b in range(B):
        # outB[dc, p, l] -> out[l, dc*128+p]
        ob = res.results[b]["outB"].astype(np.float32)
        out[b] = ob.transpose(2, 0, 1).reshape(L, C)
    return out


if __name__ == "__main__":
    rng = np.random.default_rng(0)
    x = rng.standard_normal((B, T, C), dtype=np.float32)
    w = (rng.standard_normal((C, C, K), dtype=np.float32)
         / np.sqrt(np.float32(C * K)))
    out = kernel(x, w)
    print("out", out.shape, out.dtype, float(np.abs(out).max()))


# revision 18
# speedup vs baseline: 1.0060x; 1.0060x over previous
"""Trainium2 Bass kernel for nn_ConvblockNofrills (dense_cnn).

Reference computation (per batch b, output position l, channel d):
    gate[b,l,d,k] = tanh( sum_c x[b, l+K-1, c] * weights[d, c, k] )
    out[b,l,d]    = sum_k x[b, l+k, d] * gate[b,l,d,k]
with B=8, T=4096, C=D=512, K=7, L=T-K+1=4090.

Strategy: data-parallel across the 8 NeuronCores (one batch each).
Per core everything runs in transposed (channel, position) layout:
  - gates via bf16 matmul on TensorE (fp32 PSUM accumulation)
  - tanh on ScalarE (fp32-accurate spline), output bf16 to SBUF
  - 7-tap multiply/accumulate on VectorE in bf16, per 512-col l-tile

The PE stream is the roofline (896 matmuls x 518 cyc = 193 us at
2.4 GHz); everything else exists to keep its head/tail short.
Measured constraints this is built around:
  - ~7.6 us fixed framework preamble before any kernel instruction.
  - All dma_starts drain through ONE FIFO ring at ~200-360 GB/s
    (rate depends on contiguous run length; >=4KB runs are fastest),
    so descriptors are issued on the Sync queue in exact consumption
    order: w[k=0], then x quarter-columns for all 4 channel chunks,
    then the remaining w taps, then the last x quarter.
  - The PE clock ramps (~0.9 -> 2.4 GHz) over several us and
    re-throttles after >1us idle; warmup matmuls (memset on VectorE)
    bridge from ~7.4 us until the first operands land (~12.5 us).
  - Quad 0 runs k-outer (w taps stream in one 512KB descriptor per
    ~14 us of compute); its k=0 sweep runs lt-outer/dc-inner in
    single-PSUM-bank groups so matmuls start after just w0 + one x
    quarter. Quad 1 runs dc-outer so each output chunk's store
    streams out every ~12 us instead of bunching 2MB at the end; the
    very last chunk pipelines tanh->mul->add->store per l-tile.
Host side packs/casts inputs (part of sharding) and transposes the
per-core result back to the (B, L, C) fp32 output.
"""

import numpy as np
import ml_dtypes

import sys
for _p in ("/opt/trn_rl_repo", "/root/.axon_site/_ro/trn_rl_repo"):
    if _p not in sys.path:
        sys.path.append(_p)

B, T, C, K = 8, 4096, 512, 7
L = T - K + 1  # 4090
NCORES = 8
P = 128           # partitions
DC = C // P       # 4 channel chunks
NL = 512          # l-tile (one PSUM bank of fp32)
QUAD = 4          # l-tiles per quad
NQ = 2            # quads
XQ = 1024         # x load quarter (columns)
NWARM_PRE = 4     # warmup matmuls hoisted before the entry barrier
NWARM = 6         # warmup matmuls in the body

_cache = {}


def _build():
    import concourse.bass as bass  # noqa: F401
    import concourse.mybir as mybir
    import concourse.tile as tile
    from concourse import bacc

    bf16 = mybir.dt.bfloat16
    f32 = mybir.dt.float32
    Tanh = mybir.ActivationFunctionType.Tanh

    nc = bacc.Bacc("TRN2", target_bir_lowering=False, debug=False,
                   num_devices=NCORES)

    # Instructions hoisted into the per-engine preamble (pre entry
    # barrier) after the TileContext closes: the ~7.5us framework
    # preamble then overlaps the first input DMAs and the PE clock
    # ramp. (Same entry-block insertion point bacc itself uses for the
    # kernel barrier.)
    hoist = {"sync": [], "scalar": [], "vector": [], "tensor": []}

    xT_d = nc.dram_tensor("xT", [C, T], bf16, kind="ExternalInput")
    # wB[k, p, cc, d] = weights[d, cc*128+p, k]
    wB_d = nc.dram_tensor("wB", [K, P, DC, C], bf16, kind="ExternalInput")
    # outB[dc, p, l] = out[l, dc*128+p]
    outB_d = nc.dram_tensor("outB", [DC, P, L], bf16, kind="ExternalOutput")

    with tile.TileContext(nc) as tc:
        with (
            tc.tile_pool(name="wpool", bufs=1) as wpool,
            tc.tile_pool(name="xpool", bufs=1) as xpool,
            tc.tile_pool(name="gpool", bufs=8) as gpool,
            tc.tile_pool(name="apool", bufs=6) as apool,
            tc.tile_pool(name="ppool", bufs=4) as ppool,
            tc.tile_pool(name="psum", bufs=8, space="PSUM") as psum_pool,
        ):
            wsb = wpool.tile([P, K, DC, C], bf16, name="w")
            xsb = [xpool.tile([P, T], bf16, name=f"x_{cc}")
                   for cc in range(DC)]
            warm = wpool.tile([P, NL], bf16, name="warm")

            # One FIFO DMA ring: issue in exact consumption order. The
            # first three loads are hoisted pre-barrier (issue ~5.6us).
            def load_w(k):
                return nc.sync.dma_start(wsb[:, k, :, :], wB_d.ap()[k])

            def load_x(cc, q):
                return nc.sync.dma_start(xsb[cc][:, q * XQ:(q + 1) * XQ],
                                         xT_d.ap()[cc * P:(cc + 1) * P,
                                                   q * XQ:(q + 1) * XQ])

            hoist["sync"].append(load_w(0).ins.name)
            for q in range(3):
                for cc in range(DC):
                    r = load_x(cc, q)
                    if q == 0 and cc < 2:
                        hoist["sync"].append(r.ins.name)
            for k in range(1, K):
                load_w(k)
            for cc in range(DC):
                load_x(cc, 3)

            # Warmup matmuls keep the PE clock ramping from ~5.7us
            # until real operands land. (The psum result is never read.)
            ms = nc.vector.memset(warm[:], 1.0)
            hoist["vector"].append(ms.ins.name)
            warm_ps = psum_pool.tile([P, NL], f32, tag="ps", name="warm_ps")
            for i in range(NWARM_PRE + NWARM):
                r = nc.tensor.matmul(warm_ps, warm[:, :P], warm,
                                     start=True,
                                     stop=(i == NWARM_PRE + NWARM - 1))
                if i < NWARM_PRE:
                    hoist["tensor"].append(r.ins.name)

            def mm_group(ps, k, dc, l0, nl):
                """One PSUM accumulation group: gate matmuls for
                (k, dc) over output cols [l0, l0+nl)."""
                for cc in range(DC):
                    nc.tensor.matmul(
                        ps[:, :nl],
                        wsb[:, k, cc, dc * P:(dc + 1) * P],
                        xsb[cc][:, l0 + K - 1: l0 + K - 1 + nl],
                        start=(cc == 0),
                        stop=(cc == DC - 1),
                    )

            def unit(lq, dc, k, acc, last=False):
                """Gates + tanh + per-l-tile mul/add for one (dc, k);
                returns the new accumulator tile."""
                q0 = lq * QUAD * NL
                qn = min(QUAD * NL, L - q0)
                ps = [psum_pool.tile([P, NL], f32, tag="ps",
                                     name=f"ps_{lq}_{dc}_{k}_{i}")
                      for i in range(QUAD)]
                # l-tile-outer: each PSUM group completes as early as
                # possible so the tanh chain pipelines tightly (and the
                # last unit's epilogue is short).
                for i in range(QUAD):
                    l0 = q0 + i * NL
                    nl = min(NL, L - l0)
                    mm_group(ps[i], k, dc, l0, nl)
                return tail_unit(lq, dc, k, acc, ps, last)

            def tail_unit(lq, dc, k, acc, ps, last):
                """tanh + mul(+add) per l-tile given the unit's psums."""
                q0 = lq * QUAD * NL
                g = gpool.tile([P, QUAD * NL], bf16, tag="g",
                               name=f"g_{lq}_{dc}_{k}")
                nxt = apool.tile([P, QUAD * NL], bf16, tag="acc",
                                 name=f"acc_{lq}_{dc}_{k}")
                for i in range(QUAD):
                    l0 = q0 + i * NL
                    nl = min(NL, L - l0)
                    o = i * NL
                    gs = g[:, o:o + nl]
                    nc.scalar.activation(gs, ps[i][:, :nl], Tanh)
                    xu = xsb[dc][:, l0 + k:l0 + k + nl]
                    if acc is None:
                        nc.vector.tensor_mul(nxt[:, o:o + nl], gs, xu)
                    else:
                        prod = ppool.tile([P, QUAD * NL], bf16, tag="prod",
                                          name=f"prod_{lq}_{dc}_{k}_{i}")
                        nc.vector.tensor_mul(prod[:, o:o + nl], gs, xu)
                        nc.vector.tensor_add(nxt[:, o:o + nl],
                                             acc[:, o:o + nl],
                                             prod[:, o:o + nl])
                    if last and i >= 1:
                        # stream the final unit's output out as soon as
                        # each piece's adds land: first half-quad, then
                        # per tile — the LAST store is a single 128KB
                        # tile issued right after the last add, so only
                        # its (small) data + the ~2us write receipt sit
                        # on the critical path before the exit barrier.
                        h0 = q0 + (i - 1) * NL if i == 1 else q0 + i * NL
                        oh = (i - 1) * NL if i == 1 else i * NL
                        hn = min((2 * NL if i == 1 else NL), L - h0)
                        nc.sync.dma_start(outB_d.ap()[dc, :, h0:h0 + hn],
                                          nxt[:, oh:oh + hn])
                return nxt

            def store(lq, dc, acc):
                q0 = lq * QUAD * NL
                qn = min(QUAD * NL, L - q0)
                nc.sync.dma_start(outB_d.ap()[dc, :, q0:q0 + qn],
                                  acc[:, :qn])

            # ---- Quad 0: k outer. k=0 runs lt-outer/dc-inner in
            # single-bank groups so the PE starts on minimal data.
            acc0 = [None] * DC
            ps00 = {}
            for i in range(QUAD):
                l0 = i * NL
                for dc in range(DC):
                    ps = psum_pool.tile([P, NL], f32, tag="ps",
                                        name=f"ps00_{i}_{dc}")
                    mm_group(ps, 0, dc, l0, NL)
                    ps00[(i, dc)] = ps
                    g = gpool.tile([P, NL], bf16, tag="g0",
                                   name=f"g00_{i}_{dc}")
                    nc.scalar.activation(g[:], ps[:], Tanh)
                    if acc0[dc] is None:
                        acc0[dc] = apool.tile([P, QUAD * NL], bf16,
                                              tag="acc", name=f"acc00_{dc}")
                    nc.vector.tensor_mul(acc0[dc][:, l0:l0 + NL], g[:],
                                         xsb[dc][:, l0:l0 + NL])
            for k in range(1, K):
                for dc in range(DC):
                    acc0[dc] = unit(0, dc, k, acc0[dc])
            for dc in range(DC):
                store(0, dc, acc0[dc])

            # ---- Quad 1: dc outer, k inner; stores stream per dc.
            for dc in range(DC):
                acc = None
                for k in range(K):
                    last = (dc == DC - 1 and k == K - 1)
                    acc = unit(1, dc, k, acc, last=last)
                if dc != DC - 1:
                    store(1, dc, acc)

    # Hoist the captured instructions into the entry-block preamble,
    # right after each issuing engine's preamble_end.
    blocks = nc.main_func.blocks
    entry = blocks[0]

    def move_pre_barrier(names, pend):
        order = {n: i for i, n in enumerate(names)}
        moved = []
        for blk in blocks[1:]:
            keep = []
            for inst in list(blk.instructions):
                if getattr(inst, "name", "") in order:
                    moved.append(inst)
                else:
                    keep.append(inst)
            if len(keep) != len(blk.instructions):
                blk.instructions = keep
        assert len(moved) == len(names), (len(moved), names)
        moved.sort(key=lambda i: order[i.name])
        idx = entry.instructions.index(pend) + 1
        for j, inst in enumerate(moved):
            entry.instructions.insert(idx + j, inst)

    move_pre_barrier(hoist["sync"], nc.sync.preamble_end)
    move_pre_barrier(hoist["scalar"], nc.scalar.preamble_end)
    move_pre_barrier(hoist["vector"], nc.vector.preamble_end)
    move_pre_barrier(hoist["tensor"], nc.tensor.preamble_end)

    nc.compile()
    return nc


def _prep_inputs(x, weights):
    bf = ml_dtypes.bfloat16
    # wB[k, p, cc, d] = weights[d, cc*128+p, k]
    wB = np.ascontiguousarray(
        weights.reshape(C, DC, P, K).transpose(3, 2, 1, 0)).astype(bf)
    in_maps = []
    for b in range(B):
        xT = np.ascontiguousarray(x[b].T).astype(bf)  # (C, T)
        in_maps.append({"xT": xT, "wB": wB})
    return in_maps


def kernel(x, weights):
    x = np.asarray(x, dtype=np.float32)
    weights = np.asarray(weights, dtype=np.float32)
    assert x.shape == (B, T, C) and weights.shape == (C, C, K)

    from concourse.bass_utils import run_bass_kernel_spmd

    if "nc" not in _cache:
        _cache["nc"] = _build()
    nc = _cache["nc"]

    in_maps = _prep_inputs(x, weights)
    res = run_bass_kernel_spmd(nc, in_maps, list(range(NCORES)))

    out = np.empty((B, L, C), dtype=np.float32)
    for b in range(B):
        # outB[dc, p, l] -> out[l, dc*128+p]
        ob = res.results[b]["outB"].astype(np.float32)
        out[b] = ob.transpose(2, 0, 1).reshape(L, C)
    return out


if __name__ == "__main__":
    rng = np.random.default_rng(0)
    x = rng.standard_normal((B, T, C), dtype=np.float32)
    w = (rng.standard_normal((C, C, K), dtype=np.float32)
         / np.sqrt(np.float32(C * K)))
    out = kernel(x, w)
    print("out", out.shape, out.dtype, float(np.abs(out).max()))


# revision 19
# speedup vs baseline: 1.0124x; 1.0063x over previous
"""Trainium2 Bass kernel for nn_ConvblockNofrills (dense_cnn).

Reference computation (per batch b, output position l, channel d):
    gate[b,l,d,k] = tanh( sum_c x[b, l+K-1, c] * weights[d, c, k] )
    out[b,l,d]    = sum_k x[b, l+k, d] * gate[b,l,d,k]
with B=8, T=4096, C=D=512, K=7, L=T-K+1=4090.

Strategy: data-parallel across the 8 NeuronCores (one batch each).
Per core everything runs in transposed (channel, position) layout:
  - gates via bf16 matmul on TensorE (fp32 PSUM accumulation)
  - tanh on ScalarE (fp32-accurate spline), output bf16 to SBUF
  - 7-tap multiply/accumulate on VectorE in bf16, per 512-col l-tile

The PE stream is the roofline (896 matmuls x 518 cyc = 193 us at
2.4 GHz); everything else exists to keep its head/tail short.
Measured constraints this is built around:
  - ~7.6 us fixed framework preamble before any kernel instruction.
  - All dma_starts drain through ONE FIFO ring at ~200-360 GB/s
    (rate depends on contiguous run length; >=4KB runs are fastest),
    so descriptors are issued on the Sync queue in exact consumption
    order: w[k=0], then x quarter-columns for all 4 channel chunks,
    then the remaining w taps, then the last x quarter.
  - The PE clock ramps (~0.9 -> 2.4 GHz) over several us and
    re-throttles after >1us idle; warmup matmuls (memset on VectorE)
    bridge from ~7.4 us until the first operands land (~12.5 us).
  - Quad 0 runs k-outer (w taps stream in one 512KB descriptor per
    ~14 us of compute); its k=0 sweep runs lt-outer/dc-inner in
    single-PSUM-bank groups so matmuls start after just w0 + one x
    quarter. Quad 1 runs dc-outer so each output chunk's store
    streams out every ~12 us instead of bunching 2MB at the end; the
    very last chunk pipelines tanh->mul->add->store per l-tile.
Host side packs/casts inputs (part of sharding) and transposes the
per-core result back to the (B, L, C) fp32 output.
"""

import numpy as np
import ml_dtypes

import sys
for _p in ("/opt/trn_rl_repo", "/root/.axon_site/_ro/trn_rl_repo"):
    if _p not in sys.path:
        sys.path.append(_p)

B, T, C, K = 8, 4096, 512, 7
L = T - K + 1  # 4090
NCORES = 8
P = 128           # partitions
DC = C // P       # 4 channel chunks
NL = 512          # l-tile (one PSUM bank of fp32)
QUAD = 4          # l-tiles per quad
NQ = 2            # quads
XQ = 1024         # x load quarter (columns)
NWARM_PRE = 4     # warmup matmuls hoisted before the entry barrier
NWARM = 6         # warmup matmuls in the body

_cache = {}


def _build():
    import concourse.bass as bass  # noqa: F401
    import concourse.mybir as mybir
    import concourse.tile as tile
    from concourse import bacc

    bf16 = mybir.dt.bfloat16
    f32 = mybir.dt.float32
    Tanh = mybir.ActivationFunctionType.Tanh

    nc = bacc.Bacc("TRN2", target_bir_lowering=False, debug=False,
                   num_devices=NCORES)

    # Instructions hoisted into the per-engine preamble (pre entry
    # barrier) after the TileContext closes: the ~7.5us framework
    # preamble then overlaps the first input DMAs and the PE clock
    # ramp. (Same entry-block insertion point bacc itself uses for the
    # kernel barrier.)
    hoist = {"sync": [], "scalar": [], "vector": [], "tensor": []}

    xT_d = nc.dram_tensor("xT", [C, T], bf16, kind="ExternalInput")
    # wB[k, p, cc, d] = weights[d, cc*128+p, k]
    wB_d = nc.dram_tensor("wB", [K, P, DC, C], bf16, kind="ExternalInput")
    # outB[dc, p, l] = out[l, dc*128+p]
    outB_d = nc.dram_tensor("outB", [DC, P, L], bf16, kind="ExternalOutput")

    with tile.TileContext(nc) as tc:
        with (
            tc.tile_pool(name="wpool", bufs=1) as wpool,
            tc.tile_pool(name="xpool", bufs=1) as xpool,
            tc.tile_pool(name="gpool", bufs=8) as gpool,
            tc.tile_pool(name="apool", bufs=6) as apool,
            tc.tile_pool(name="ppool", bufs=4) as ppool,
            tc.tile_pool(name="psum", bufs=8, space="PSUM") as psum_pool,
        ):
            wsb = wpool.tile([P, K, DC, C], bf16, name="w")
            xsb = [xpool.tile([P, T], bf16, name=f"x_{cc}")
                   for cc in range(DC)]
            warm = wpool.tile([P, NL], bf16, name="warm")

            # One FIFO DMA ring: issue in exact consumption order. The
            # first three loads are hoisted pre-barrier (issue ~5.6us).
            def load_w(k):
                return nc.sync.dma_start(wsb[:, k, :, :], wB_d.ap()[k])

            def load_x(cc, q):
                return nc.sync.dma_start(xsb[cc][:, q * XQ:(q + 1) * XQ],
                                         xT_d.ap()[cc * P:(cc + 1) * P,
                                                   q * XQ:(q + 1) * XQ])

            # Hoist only two loads: a third delays the Sync engine's
            # entry-barrier arrival (~0.6us/issue), which holds back
            # every body-issued load by the same amount.
            hoist["sync"].append(load_w(0).ins.name)
            for q in range(3):
                for cc in range(DC):
                    r = load_x(cc, q)
                    if q == 0 and cc < 1:
                        hoist["sync"].append(r.ins.name)
            for k in range(1, K):
                load_w(k)
            for cc in range(DC):
                load_x(cc, 3)

            # Warmup matmuls keep the PE clock ramping from ~5.7us
            # until real operands land. (The psum result is never read.)
            ms = nc.vector.memset(warm[:], 1.0)
            hoist["vector"].append(ms.ins.name)
            warm_ps = psum_pool.tile([P, NL], f32, tag="ps", name="warm_ps")
            for i in range(NWARM_PRE + NWARM):
                r = nc.tensor.matmul(warm_ps, warm[:, :P], warm,
                                     start=True,
                                     stop=(i == NWARM_PRE + NWARM - 1))
                if i < NWARM_PRE:
                    hoist["tensor"].append(r.ins.name)

            def mm_group(ps, k, dc, l0, nl):
                """One PSUM accumulation group: gate matmuls for
                (k, dc) over output cols [l0, l0+nl)."""
                for cc in range(DC):
                    nc.tensor.matmul(
                        ps[:, :nl],
                        wsb[:, k, cc, dc * P:(dc + 1) * P],
                        xsb[cc][:, l0 + K - 1: l0 + K - 1 + nl],
                        start=(cc == 0),
                        stop=(cc == DC - 1),
                    )

            def unit(lq, dc, k, acc, last=False):
                """Gates + tanh + per-l-tile mul/add for one (dc, k);
                returns the new accumulator tile."""
                q0 = lq * QUAD * NL
                qn = min(QUAD * NL, L - q0)
                ps = [psum_pool.tile([P, NL], f32, tag="ps",
                                     name=f"ps_{lq}_{dc}_{k}_{i}")
                      for i in range(QUAD)]
                # l-tile-outer: each PSUM group completes as early as
                # possible so the tanh chain pipelines tightly (and the
                # last unit's epilogue is short).
                for i in range(QUAD):
                    l0 = q0 + i * NL
                    nl = min(NL, L - l0)
                    mm_group(ps[i], k, dc, l0, nl)
                return tail_unit(lq, dc, k, acc, ps, last)

            def tail_unit(lq, dc, k, acc, ps, last):
                """tanh + mul(+add) per l-tile given the unit's psums."""
                q0 = lq * QUAD * NL
                g = gpool.tile([P, QUAD * NL], bf16, tag="g",
                               name=f"g_{lq}_{dc}_{k}")
                nxt = apool.tile([P, QUAD * NL], bf16, tag="acc",
                                 name=f"acc_{lq}_{dc}_{k}")
                for i in range(QUAD):
                    l0 = q0 + i * NL
                    nl = min(NL, L - l0)
                    o = i * NL
                    gs = g[:, o:o + nl]
                    nc.scalar.activation(gs, ps[i][:, :nl], Tanh)
                    xu = xsb[dc][:, l0 + k:l0 + k + nl]
                    if acc is None:
                        nc.vector.tensor_mul(nxt[:, o:o + nl], gs, xu)
                    else:
                        prod = ppool.tile([P, QUAD * NL], bf16, tag="prod",
                                          name=f"prod_{lq}_{dc}_{k}_{i}")
                        nc.vector.tensor_mul(prod[:, o:o + nl], gs, xu)
                        nc.vector.tensor_add(nxt[:, o:o + nl],
                                             acc[:, o:o + nl],
                                             prod[:, o:o + nl])
                    if last and i >= 1:
                        # stream the final unit's output out as soon as
                        # each piece's adds land: first half-quad, then
                        # per tile — the LAST store is a single 128KB
                        # tile issued right after the last add, so only
                        # its (small) data + the ~2us write receipt sit
                        # on the critical path before the exit barrier.
                        h0 = q0 + (i - 1) * NL if i == 1 else q0 + i * NL
                        oh = (i - 1) * NL if i == 1 else i * NL
                        hn = min((2 * NL if i == 1 else NL), L - h0)
                        nc.sync.dma_start(outB_d.ap()[dc, :, h0:h0 + hn],
                                          nxt[:, oh:oh + hn])
                return nxt

            def store(lq, dc, acc):
                q0 = lq * QUAD * NL
                qn = min(QUAD * NL, L - q0)
                nc.sync.dma_start(outB_d.ap()[dc, :, q0:q0 + qn],
                                  acc[:, :qn])

            # ---- Quad 0: k outer. k=0 runs lt-outer/dc-inner in
            # single-bank groups so the PE starts on minimal data.
            acc0 = [None] * DC
            ps00 = {}
            for i in range(QUAD):
                l0 = i * NL
                for dc in range(DC):
                    ps = psum_pool.tile([P, NL], f32, tag="ps",
                                        name=f"ps00_{i}_{dc}")
                    mm_group(ps, 0, dc, l0, NL)
                    ps00[(i, dc)] = ps
                    g = gpool.tile([P, NL], bf16, tag="g0",
                                   name=f"g00_{i}_{dc}")
                    nc.scalar.activation(g[:], ps[:], Tanh)
                    if acc0[dc] is None:
                        acc0[dc] = apool.tile([P, QUAD * NL], bf16,
                                              tag="acc", name=f"acc00_{dc}")
                    nc.vector.tensor_mul(acc0[dc][:, l0:l0 + NL], g[:],
                                         xsb[dc][:, l0:l0 + NL])
            for k in range(1, K):
                for dc in range(DC):
                    acc0[dc] = unit(0, dc, k, acc0[dc])
            for dc in range(DC):
                store(0, dc, acc0[dc])

            # ---- Quad 1: dc outer, k inner; stores stream per dc.
            for dc in range(DC):
                acc = None
                for k in range(K):
                    last = (dc == DC - 1 and k == K - 1)
                    acc = unit(1, dc, k, acc, last=last)
                if dc != DC - 1:
                    store(1, dc, acc)

    # Hoist the captured instructions into the entry-block preamble,
    # right after each issuing engine's preamble_end.
    blocks = nc.main_func.blocks
    entry = blocks[0]

    def move_pre_barrier(names, pend):
        order = {n: i for i, n in enumerate(names)}
        moved = []
        for blk in blocks[1:]:
            keep = []
            for inst in list(blk.instructions):
                if getattr(inst, "name", "") in order:
                    moved.append(inst)
                else:
                    keep.append(inst)
            if len(keep) != len(blk.instructions):
                blk.instructions = keep
        assert len(moved) == len(names), (len(moved), names)
        moved.sort(key=lambda i: order[i.name])
        idx = entry.instructions.index(pend) + 1
        for j, inst in enumerate(moved):
            entry.instructions.insert(idx + j, inst)

    move_pre_barrier(hoist["sync"], nc.sync.preamble_end)
    move_pre_barrier(hoist["scalar"], nc.scalar.preamble_end)
    move_pre_barrier(hoist["vector"], nc.vector.preamble_end)
    move_pre_barrier(hoist["tensor"], nc.tensor.preamble_end)

    nc.compile()
    return nc


def _prep_inputs(x, weights):
    bf = ml_dtypes.bfloat16
    # wB[k, p, cc, d] = weights[d, cc*128+p, k]
    wB = np.ascontiguousarray(
        weights.reshape(C, DC, P, K).transpose(3, 2, 1, 0)).astype(bf)
    in_maps = []
    for b in range(B):
        xT = np.ascontiguousarray(x[b].T).astype(bf)  # (C, T)
        in_maps.append({"xT": xT, "wB": wB})
    return in_maps


def kernel(x, weights):
    x = np.asarray(x, dtype=np.float32)
    weights = np.asarray(weights, dtype=np.float32)
    assert x.shape == (B, T, C) and weights.shape == (C, C, K)

    from concourse.bass_utils import run_bass_kernel_spmd

    if "nc" not in _cache:
        _cache["nc"] = _build()
    nc = _cache["nc"]

    in_maps = _prep_inputs(x, weights)
    res = run_bass_kernel_spmd(nc, in_maps, list(range(NCORES)))

    out = np.empty((B, L, C), dtype=np.float32)
    for b in range(B):
        # outB[dc, p, l] -> out[l, dc*128+p]
        ob = res.results[b]["outB"].astype(np.float32)
        out[b] = ob.transpose(2, 0, 1).reshape(L, C)
    return out


if __name__ == "__main__":
    rng = np.random.default_rng(0)
    x = rng.standard_normal((B, T, C), dtype=np.float32)
    w = (rng.standard_normal((C, C, K), dtype=np.float32)
         / np.sqrt(np.float32(C * K)))
    out = kernel(x, w)
    print("out", out.shape, out.dtype, float(np.abs(out).max()))


# revision 20
# speedup vs baseline: 1.0136x; 1.0012x over previous
"""Trainium2 Bass kernel for nn_ConvblockNofrills (dense_cnn).

Reference computation (per batch b, output position l, channel d):
    gate[b,l,d,k] = tanh( sum_c x[b, l+K-1, c] * weights[d, c, k] )
    out[b,l,d]    = sum_k x[b, l+k, d] * gate[b,l,d,k]
with B=8, T=4096, C=D=512, K=7, L=T-K+1=4090.

Strategy: data-parallel across the 8 NeuronCores (one batch each).
Per core everything runs in transposed (channel, position) layout:
  - gates via bf16 matmul on TensorE (fp32 PSUM accumulation)
  - tanh on ScalarE (fp32-accurate spline), output bf16 to SBUF
  - 7-tap multiply/accumulate on VectorE in bf16, per 512-col l-tile

The PE stream is the roofline (896 matmuls x 518 cyc = 193 us at
2.4 GHz); everything else exists to keep its head/tail short.
Measured constraints this is built around:
  - ~7.6 us fixed framework preamble before any kernel instruction.
  - All dma_starts drain through ONE FIFO ring at ~200-360 GB/s
    (rate depends on contiguous run length; >=4KB runs are fastest),
    so descriptors are issued on the Sync queue in exact consumption
    order: w[k=0], then x quarter-columns for all 4 channel chunks,
    then the remaining w taps, then the last x quarter.
  - The PE clock ramps (~0.9 -> 2.4 GHz) over several us and
    re-throttles after >1us idle; warmup matmuls (memset on VectorE)
    bridge from ~7.4 us until the first operands land (~12.5 us).
  - Quad 0 runs k-outer (w taps stream in one 512KB descriptor per
    ~14 us of compute); its k=0 sweep runs lt-outer/dc-inner in
    single-PSUM-bank groups so matmuls start after just w0 + one x
    quarter. Quad 1 runs dc-outer so each output chunk's store
    streams out every ~12 us instead of bunching 2MB at the end; the
    very last chunk pipelines tanh->mul->add->store per l-tile.
Host side packs/casts inputs (part of sharding) and transposes the
per-core result back to the (B, L, C) fp32 output.
"""

import numpy as np
import ml_dtypes

import sys
for _p in ("/opt/trn_rl_repo", "/root/.axon_site/_ro/trn_rl_repo"):
    if _p not in sys.path:
        sys.path.append(_p)

B, T, C, K = 8, 4096, 512, 7
L = T - K + 1  # 4090
NCORES = 8
P = 128           # partitions
DC = C // P       # 4 channel chunks
NL = 512          # l-tile (one PSUM bank of fp32)
QUAD = 4          # l-tiles per quad
NQ = 2            # quads
XQ = 1024         # x load quarter (columns)
NWARM_PRE = 3     # warmup matmuls hoisted before the entry barrier
NWARM = 7         # warmup matmuls in the body

_cache = {}


def _build():
    import concourse.bass as bass  # noqa: F401
    import concourse.mybir as mybir
    import concourse.tile as tile
    from concourse import bacc

    bf16 = mybir.dt.bfloat16
    f32 = mybir.dt.float32
    Tanh = mybir.ActivationFunctionType.Tanh

    nc = bacc.Bacc("TRN2", target_bir_lowering=False, debug=False,
                   num_devices=NCORES)

    # Instructions hoisted into the per-engine preamble (pre entry
    # barrier) after the TileContext closes: the ~7.5us framework
    # preamble then overlaps the first input DMAs and the PE clock
    # ramp. (Same entry-block insertion point bacc itself uses for the
    # kernel barrier.)
    hoist = {"sync": [], "scalar": [], "vector": [], "tensor": []}

    xT_d = nc.dram_tensor("xT", [C, T], bf16, kind="ExternalInput")
    # wB[k, p, cc, d] = weights[d, cc*128+p, k]
    wB_d = nc.dram_tensor("wB", [K, P, DC, C], bf16, kind="ExternalInput")
    # outB[dc, p, l] = out[l, dc*128+p]
    outB_d = nc.dram_tensor("outB", [DC, P, L], bf16, kind="ExternalOutput")

    with tile.TileContext(nc) as tc:
        with (
            tc.tile_pool(name="wpool", bufs=1) as wpool,
            tc.tile_pool(name="xpool", bufs=1) as xpool,
            tc.tile_pool(name="gpool", bufs=8) as gpool,
            tc.tile_pool(name="apool", bufs=6) as apool,
            tc.tile_pool(name="ppool", bufs=4) as ppool,
            tc.tile_pool(name="psum", bufs=8, space="PSUM") as psum_pool,
        ):
            wsb = wpool.tile([P, K, DC, C], bf16, name="w")
            xsb = [xpool.tile([P, T], bf16, name=f"x_{cc}")
                   for cc in range(DC)]
            warm = wpool.tile([P, NL], bf16, name="warm")

            # One FIFO DMA ring: issue in exact consumption order. The
            # first three loads are hoisted pre-barrier (issue ~5.6us).
            def load_w(k):
                return nc.sync.dma_start(wsb[:, k, :, :], wB_d.ap()[k])

            def load_x(cc, q):
                return nc.sync.dma_start(xsb[cc][:, q * XQ:(q + 1) * XQ],
                                         xT_d.ap()[cc * P:(cc + 1) * P,
                                                   q * XQ:(q + 1) * XQ])

            # Hoist only two loads: a third delays the Sync engine's
            # entry-barrier arrival (~0.6us/issue), which holds back
            # every body-issued load by the same amount.
            hoist["sync"].append(load_w(0).ins.name)
            for q in range(3):
                for cc in range(DC):
                    r = load_x(cc, q)
                    if q == 0 and cc < 1:
                        hoist["sync"].append(r.ins.name)
            for k in range(1, K):
                load_w(k)
            for cc in range(DC):
                load_x(cc, 3)

            # Warmup matmuls keep the PE clock ramping from ~5.7us
            # until real operands land. (The psum result is never read.)
            ms = nc.vector.memset(warm[:], 1.0)
            hoist["vector"].append(ms.ins.name)
            warm_ps = psum_pool.tile([P, NL], f32, tag="ps", name="warm_ps")
            for i in range(NWARM_PRE + NWARM):
                r = nc.tensor.matmul(warm_ps, warm[:, :P], warm,
                                     start=True,
                                     stop=(i == NWARM_PRE + NWARM - 1))
                if i < NWARM_PRE:
                    hoist["tensor"].append(r.ins.name)

            def mm_group(ps, k, dc, l0, nl):
                """One PSUM accumulation group: gate matmuls for
                (k, dc) over output cols [l0, l0+nl)."""
                for cc in range(DC):
                    nc.tensor.matmul(
                        ps[:, :nl],
                        wsb[:, k, cc, dc * P:(dc + 1) * P],
                        xsb[cc][:, l0 + K - 1: l0 + K - 1 + nl],
                        start=(cc == 0),
                        stop=(cc == DC - 1),
                    )

            def unit(lq, dc, k, acc, last=False):
                """Gates + tanh + per-l-tile mul/add for one (dc, k);
                returns the new accumulator tile."""
                q0 = lq * QUAD * NL
                qn = min(QUAD * NL, L - q0)
                ps = [psum_pool.tile([P, NL], f32, tag="ps",
                                     name=f"ps_{lq}_{dc}_{k}_{i}")
                      for i in range(QUAD)]
                # l-tile-outer: each PSUM group completes as early as
                # possible so the tanh chain pipelines tightly (and the
                # last unit's epilogue is short).
                for i in range(QUAD):
                    l0 = q0 + i * NL
                    nl = min(NL, L - l0)
                    mm_group(ps[i], k, dc, l0, nl)
                return tail_unit(lq, dc, k, acc, ps, last)

            def tail_unit(lq, dc, k, acc, ps, last):
                """tanh + mul(+add) per l-tile given the unit's psums."""
                q0 = lq * QUAD * NL
                g = gpool.tile([P, QUAD * NL], bf16, tag="g",
                               name=f"g_{lq}_{dc}_{k}")
                nxt = apool.tile([P, QUAD * NL], bf16, tag="acc",
                                 name=f"acc_{lq}_{dc}_{k}")
                for i in range(QUAD):
                    l0 = q0 + i * NL
                    nl = min(NL, L - l0)
                    o = i * NL
                    gs = g[:, o:o + nl]
                    nc.scalar.activation(gs, ps[i][:, :nl], Tanh)
                    xu = xsb[dc][:, l0 + k:l0 + k + nl]
                    if acc is None:
                        nc.vector.tensor_mul(nxt[:, o:o + nl], gs, xu)
                    else:
                        prod = ppool.tile([P, QUAD * NL], bf16, tag="prod",
                                          name=f"prod_{lq}_{dc}_{k}_{i}")
                        nc.vector.tensor_mul(prod[:, o:o + nl], gs, xu)
                        nc.vector.tensor_add(nxt[:, o:o + nl],
                                             acc[:, o:o + nl],
                                             prod[:, o:o + nl])
                    if last and i >= 1:
                        # stream the final unit's output out as soon as
                        # each piece's adds land: first half-quad, then
                        # per tile — the LAST store is a single 128KB
                        # tile issued right after the last add, so only
                        # its (small) data + the ~2us write receipt sit
                        # on the critical path before the exit barrier.
                        h0 = q0 + (i - 1) * NL if i == 1 else q0 + i * NL
                        oh = (i - 1) * NL if i == 1 else i * NL
                        hn = min((2 * NL if i == 1 else NL), L - h0)
                        nc.sync.dma_start(outB_d.ap()[dc, :, h0:h0 + hn],
                                          nxt[:, oh:oh + hn])
                return nxt

            def store(lq, dc, acc):
                q0 = lq * QUAD * NL
                qn = min(QUAD * NL, L - q0)
                nc.sync.dma_start(outB_d.ap()[dc, :, q0:q0 + qn],
                                  acc[:, :qn])

            # ---- Quad 0: k outer. k=0 runs lt-outer/dc-inner in
            # single-bank groups so the PE starts on minimal data.
            acc0 = [None] * DC
            ps00 = {}
            for i in range(QUAD):
                l0 = i * NL
                for dc in range(DC):
                    ps = psum_pool.tile([P, NL], f32, tag="ps",
                                        name=f"ps00_{i}_{dc}")
                    mm_group(ps, 0, dc, l0, NL)
                    ps00[(i, dc)] = ps
                    g = gpool.tile([P, NL], bf16, tag="g0",
                                   name=f"g00_{i}_{dc}")
                    nc.scalar.activation(g[:], ps[:], Tanh)
                    if acc0[dc] is None:
                        acc0[dc] = apool.tile([P, QUAD * NL], bf16,
                                              tag="acc", name=f"acc00_{dc}")
                    nc.vector.tensor_mul(acc0[dc][:, l0:l0 + NL], g[:],
                                         xsb[dc][:, l0:l0 + NL])
            for k in range(1, K):
                for dc in range(DC):
                    acc0[dc] = unit(0, dc, k, acc0[dc])
            for dc in range(DC):
                store(0, dc, acc0[dc])

            # ---- Quad 1: dc outer, k inner; stores stream per dc.
            for dc in range(DC):
                acc = None
                for k in range(K):
                    last = (dc == DC - 1 and k == K - 1)
                    acc = unit(1, dc, k, acc, last=last)
                if dc != DC - 1:
                    store(1, dc, acc)

    # Hoist the captured instructions into the entry-block preamble,
    # right after each issuing engine's preamble_end.
    blocks = nc.main_func.blocks
    entry = blocks[0]

    def move_pre_barrier(names, pend):
        order = {n: i for i, n in enumerate(names)}
        moved = []
        for blk in blocks[1:]:
            keep = []
            for inst in list(blk.instructions):
                if getattr(inst, "name", "") in order:
                    moved.append(inst)
                else:
                    keep.append(inst)
            if len(keep) != len(blk.instructions):
                blk.instructions = keep
        assert len(moved) == len(names), (len(moved), names)
        moved.sort(key=lambda i: order[i.name])
        idx = entry.instructions.index(pend) + 1
        for j, inst in enumerate(moved):
            entry.instructions.insert(idx + j, inst)

    move_pre_barrier(hoist["sync"], nc.sync.preamble_end)
    move_pre_barrier(hoist["scalar"], nc.scalar.preamble_end)
    move_pre_barrier(hoist["vector"], nc.vector.preamble_end)
    move_pre_barrier(hoist["tensor"], nc.tensor.preamble_end)

    nc.compile()
    return nc


def _prep_inputs(x, weights):
    bf = ml_dtypes.bfloat16
    # wB[k, p, cc, d] = weights[d, cc*128+p, k]
    wB = np.ascontiguousarray(
        weights.reshape(C, DC, P, K).transpose(3, 2, 1, 0)).astype(bf)
    in_maps = []
    for b in range(B):
        xT = np.ascontiguousarray(x[b].T).astype(bf)  # (C, T)
        in_maps.append({"xT": xT, "wB": wB})
    return in_maps


def kernel(x, weights):
    x = np.asarray(x, dtype=np.float32)
    weights = np.asarray(weights, dtype=np.float32)
    assert x.shape == (B, T, C) and weights.shape == (C, C, K)

    from concourse.bass_utils import run_bass_kernel_spmd

    if "nc" not in _cache:
        _cache["nc"] = _build()
    nc = _cache["nc"]

    in_maps = _prep_inputs(x, weights)
    res = run_bass_kernel_spmd(nc, in_maps, list(range(NCORES)))

    out = np.empty((B, L, C), dtype=np.float32)
    for b in range(B):
        # outB[dc, p, l] -> out[l, dc*128+p]
        ob = res.results[b]["outB"].astype(np.float32)
        out[b] = ob.transpose(2, 0, 1).reshape(L, C)
    return out


if __name__ == "__main__":
    rng = np.random.default_rng(0)
    x = rng.standard_normal((B, T, C), dtype=np.float32)
    w = (rng.standard_normal((C, C, K), dtype=np.float32)
         / np.sqrt(np.float32(C * K)))
    out = kernel(x, w)
    print("out", out.shape, out.dtype, float(np.abs(out).max()))
